# revision 1
# baseline (speedup 1.0000x reference)
"""Per-pixel dynamic 7x7 filtering (BaseTextureDiffusion._diffusion_step)
on 8 Trainium2 NeuronCores.

out[b,c,h,w] = sum_k weights[b,c,k,h,w] * pad_edge(latent)[b,c,h+i,w+j],
k = i*7+j.

Sharding: the 48 (b,c) planes are independent -> 6 planes per core.
Latent is replicate-padded on host (tiny) so the device kernel does no
edge handling.

Device layout per core: partition dim = image rows (2 blocks of 128),
free dim = (plane, col) -> 1536 elems per partition per op.  For each row
block, row-shifted padded-latent tiles are loaded just-in-time per tap
group (col shifts become free-dim slices; a one-element-shifted copy
keeps odd col shifts 4B-aligned for the DVE 2x fp16 mode).  Compute is
49 tensor_mul products per block on the vector engine (fp16, 2x mode),
accumulated hierarchically: fp16 partial per 10-tap group, group partials
summed into an f32 total (keeps scale-relative error ~6e-4 while the
856 MB of f32 inputs are shipped as fp16 -> half the HBM traffic); the
group-0 partial seeds the f32 total directly through a mixed-dtype add.
Weight loads are split into per-tap slice DMAs: Tile's subtile dependency
tracking then lets each tensor_mul start as soon as its own tap's weights
land instead of stalling on a whole 8 MB group load (-12%).  Measured
~163-170 us/core/invocation, DVE-throughput-bound at the DVE busy floor
(all off-DVE offloads checked and rejected: gpsimd fp16 tensor ops ~11x
slower than DVE and mixed-dtype merges on gpsimd also regress, ScalarE
has no 2-tensor op, PE reduces over partitions only, DMA-accumulate is
fabric-bound).  gpsimd/f32 fallbacks retained behind flags.
"""

import numpy as np

B, C, H, W = 2, 24, 256, 256
R = 7
PAD = R // 2
NCORES = 8
PLANES = B * C  # 48
PPC = PLANES // NCORES  # 6 planes per core
HP = H + 2 * PAD  # 262
WP = W + 2 * PAD  # 262
KG = 5  # taps per weight DMA group (f32)
DTYPE = "f16"  # compute dtype for weights/latent ('f32' | 'f16')
GPS_EVERY = 1  # (f32 path) 1-in-N accumulate ops go to DVE; rest to gpsimd
STAGGERED = False  # staggered sem reset on the For_i timing loop
ACC32 = True  # fp16 group partials accumulated into an f32 total
GPS_MERGE = False  # route the f32 group-merge adds to gpsimd
SPLIT_WG = True  # per-tap weight DMAs (finer-grained overlap via subtile deps)

_cache = {}


def _split_multi_waits(nc, max_waits: int = 1):
    """walrus CoreV3 codegen in this container rejects instructions carrying
    more than one sync wait ('Too many sync wait commands').  Legalize the
    module by hoisting extra waits onto same-engine NoOps inserted directly
    before the instruction (engine stalls at the nop first — semantics
    preserved, the instruction still executes only after all conditions)."""
    import concourse.mybir as mybir

    cnt = 0
    for f in nc.m.functions:
        for b in f.blocks:
            changed = False
            new_insts = []
            for inst in b.instructions:
                si = inst.sync_info
                if si is not None and len(si.on_wait) > max_waits:
                    waits = list(si.on_wait)
                    upds = list(si.on_update)
                    chunks = [
                        waits[i : i + max_waits]
                        for i in range(0, len(waits), max_waits)
                    ]
                    for chunk in chunks[:-1]:
                        nop = mybir.InstNoOp(
                            name=f"ws_nop_{cnt}", ins=[], outs=[]
                        )
                        cnt += 1
                        nop.engine = inst.engine
                        nop.sync_info = mybir.SyncInfo(
                            on_wait=chunk, on_update=[]
                        )
                        new_insts.append(nop)
                    inst.sync_info = mybir.SyncInfo(
                        on_wait=chunks[-1], on_update=upds
                    )
                    changed = True
                new_insts.append(inst)
            if changed:
                b.instructions = new_insts


def build_nc(
    reps: int = 1,
    dtype: str = DTYPE,
    gps_every: int = GPS_EVERY,
    loop_reps: int | None = None,
    skip_compute: bool = False,
):
    """Build the per-core Bass program (SPMD; all cores run the same NEFF).

    dtype: 'f32' or 'f16' for weights/latent/accumulation (output always f32).
    gps_every: every gps_every-th tap's accumulate goes to DVE (accA); the
    rest go to gpsimd (accB).  0 = everything on DVE.
    loop_reps: if set, wrap ONE rep body in a hardware For_i loop with this
    trip count (constant NEFF size for any count; used for timing).
    """
    import concourse.bass as bass
    import concourse.mybir as mybir
    from concourse.tile import TileContext

    f16 = dtype == "f16"
    dt = mybir.dt.float16 if f16 else mybir.dt.float32
    dto = mybir.dt.float32
    kg = 10 if f16 else KG

    nc = bass.Bass("TRN2", target_bir_lowering=False, debug=False, num_devices=NCORES)
    # Weights are pre-transposed on host to [row, k, plane, col] so each
    # (row-block, tap-group) DMA is contiguous per partition (2-dim AP).
    wt_r = nc.dram_tensor("wt", [H, R * R, PPC, W], dt, kind="ExternalInput").ap()
    lp = nc.dram_tensor("lp", [PPC, HP, WP], dt, kind="ExternalInput").ap()
    out = nc.dram_tensor("out", [PPC, H, W], dto, kind="ExternalOutput").ap()

    # Rows on the partition dim.
    lp_r = lp.rearrange("pl r d -> r pl d")  # [262, 6, 262]
    out_r = out.rearrange("pl r c -> r pl c")  # [256, 6, 256]

    def route_add(k):
        # >0: accA (DVE) for every Nth tap, rest gpsimd.  <0: reversed.
        if gps_every == 0:
            return "A"
        if gps_every > 0:
            return "A" if k % gps_every == 0 else "B"
        return "B" if k % (-gps_every) == 0 else "A"

    with TileContext(nc) as tc:
        with tc.tile_pool(name="pool", bufs=1) as pool:

            def rep_body(rep):
                for blk in range(H // 128):
                    r0 = blk * 128
                    # Row-shifted padded-latent tiles, loaded lazily right
                    # before the first tap group that needs them so compute
                    # starts as soon as the first group's tiles land.  For
                    # f16, DVE 2x mode needs 4B-aligned slices, so odd col
                    # shifts read a copy pre-shifted by one element.
                    rs = {}  # i -> even-aligned tile for row shift i
                    rso = {}  # i -> same, shifted one col left

                    def need_row(i):
                        if i in rs:
                            return
                        t = pool.tile(
                            [128, PPC, WP], dt,
                            name=f"rs_{rep}_{blk}_{i}", tag=f"rs{i}", bufs=2,
                        )
                        nc.sync.dma_start(out=t[:], in_=lp_r[r0 + i : r0 + i + 128])
                        rs[i] = t
                        if f16:
                            to = pool.tile(
                                [128, PPC, WP], dt,
                                name=f"rso_{rep}_{blk}_{i}", tag=f"rso{i}", bufs=2,
                            )
                            nc.sync.dma_start(
                                out=to[:, :, 0 : WP - 1],
                                in_=lp_r[r0 + i : r0 + i + 128, :, 1:WP],
                            )
                            rso[i] = to

                    if not ACC32:
                        accA = pool.tile(
                            [128, PPC, W], dt,
                            name=f"accA_{rep}_{blk}", tag="accA", bufs=1,
                        )
                        accB = pool.tile(
                            [128, PPC, W], dt,
                            name=f"accB_{rep}_{blk}", tag="accB", bufs=1,
                        )
                    acc32 = pool.tile(
                        [128, PPC, W], dto, name=f"acc32_{rep}_{blk}", tag="acc32",
                        bufs=1,
                    )
                    if skip_compute:
                        nc.gpsimd.memset(acc32[:], 0.0)
                    n_groups = (R * R + kg - 1) // kg
                    for g in range(n_groups):
                        g0, g1 = g * kg, min(g * kg + kg, R * R)
                        for i in sorted({k // R for k in range(g0, g1)}):
                            need_row(i)
                        wg = pool.tile(
                            [128, g1 - g0, PPC, W], dt,
                            name=f"wg_{rep}_{blk}_{g0}", tag="wg", bufs=2,
                        )
                        if SPLIT_WG:
                            # per-tap slice DMAs: subtile deps let each mult
                            # start as soon as its own tap's weights land
                            for k in range(g0, g1):
                                nc.sync.dma_start(
                                    out=wg[:, k - g0],
                                    in_=wt_r[r0 : r0 + 128, k],
                                )
                        else:
                            nc.sync.dma_start(
                                out=wg[:], in_=wt_r[r0 : r0 + 128, g0:g1]
                            )
                        gacc = (
                            pool.tile(
                                [128, PPC, W], dt,
                                name=f"gacc_{rep}_{blk}_{g0}", tag="gacc", bufs=2,
                            )
                            if ACC32
                            else None
                        )
                        for k in range(g0, g1):
                            if skip_compute:
                                continue
                            i, j = divmod(k, R)
                            if f16 and (j % 2 == 1):
                                x = rso[i][:, :, j - 1 : j - 1 + W]
                            else:
                                x = rs[i][:, :, j : j + W]
                            w = wg[:, k - g0]
                            if ACC32:
                                # fp16 partial per group, f32 running total
                                if k == g0:
                                    nc.vector.tensor_mul(gacc[:], w, x)
                                else:
                                    prod = pool.tile(
                                        [128, PPC, W], dt,
                                        name=f"prod_{rep}_{blk}_{k}", tag="prod",
                                        bufs=3,
                                    )
                                    nc.vector.tensor_mul(prod[:], w, x)
                                    if g == 0 and k == g1 - 1:
                                        # last tap of group 0 seeds the f32
                                        # total directly (skips a copy)
                                        nc.vector.tensor_add(
                                            acc32[:], gacc[:], prod[:]
                                        )
                                    else:
                                        nc.vector.tensor_add(
                                            gacc[:], gacc[:], prod[:]
                                        )
                            elif k == 0:
                                nc.vector.tensor_mul(accA[:], w, x)
                            elif k == 1:
                                nc.vector.tensor_mul(accB[:], w, x)
                            else:
                                prod = pool.tile(
                                    [128, PPC, W], dt,
                                    name=f"prod_{rep}_{blk}_{k}", tag="prod", bufs=3,
                                )
                                nc.vector.tensor_mul(prod[:], w, x)
                                if route_add(k) == "A":
                                    nc.vector.tensor_add(accA[:], accA[:], prod[:])
                                else:
                                    nc.gpsimd.tensor_add(accB[:], accB[:], prod[:])
                        if ACC32 and not skip_compute and g > 0:
                            if GPS_MERGE:
                                nc.gpsimd.tensor_add(acc32[:], acc32[:], gacc[:])
                            else:
                                nc.vector.tensor_add(acc32[:], acc32[:], gacc[:])
                    if ACC32:
                        nc.sync.dma_start(out=out_r[r0 : r0 + 128], in_=acc32[:])
                    elif f16:
                        nc.vector.tensor_add(acc32[:], accA[:], accB[:])
                        nc.sync.dma_start(out=out_r[r0 : r0 + 128], in_=acc32[:])
                    else:
                        nc.vector.tensor_add(accA[:], accA[:], accB[:])
                        nc.sync.dma_start(out=out_r[r0 : r0 + 128], in_=accA[:])

            if loop_reps is not None:
                with tc.For_i(0, loop_reps, 1, staggered_reset=STAGGERED):
                    rep_body(0)
            else:
                for rep in range(reps):
                    rep_body(rep)
    _split_multi_waits(nc)
    return nc


def _prep_inputs(latent, weights, dtype: str = DTYPE):
    npdt = np.float16 if dtype == "f16" else np.float32
    lat = np.asarray(latent, dtype=np.float32).reshape(PLANES, H, W)
    wts = np.asarray(weights, dtype=np.float32).reshape(PLANES, R * R, H, W)
    lpad = np.pad(lat, ((0, 0), (PAD, PAD), (PAD, PAD)), mode="edge").astype(npdt)
    in_maps = []
    for c in range(NCORES):
        wc = wts[c * PPC : (c + 1) * PPC]  # [6, 49, 256, 256]
        # -> [row, k, plane, col] so device DMAs are contiguous per row.
        wc = np.ascontiguousarray(wc.transpose(2, 1, 0, 3).astype(npdt))
        in_maps.append(
            {
                "wt": wc,
                "lp": np.ascontiguousarray(lpad[c * PPC : (c + 1) * PPC]),
            }
        )
    return in_maps


def _get_runner():
    """Build the Bass program and ONE sharded jit executable, cached for the
    process.  Repeated kernel() calls reuse the same loaded executable —
    creating a fresh jit per call (as run_bass_kernel_spmd does) loads a new
    executable each time and can wedge the device on the second call."""
    if "runner" in _cache:
        return _cache["runner"]

    import jax
    import concourse.mybir as mybir
    from concourse import bass2jax
    from jax.experimental.shard_map import shard_map
    from jax.sharding import Mesh, NamedSharding, PartitionSpec

    bass2jax.install_neuronx_cc_hook()
    nc = build_nc(reps=1)

    partition_name = nc.partition_id_tensor.name if nc.partition_id_tensor else None
    in_names, out_names, out_avals, zero_outs = [], [], [], []
    for alloc in nc.m.functions[0].allocations:
        if not isinstance(alloc, mybir.MemoryLocationSet):
            continue
        name = alloc.memorylocations[0].name
        if alloc.kind == "ExternalInput":
            if name != partition_name:
                in_names.append(name)
        elif alloc.kind == "ExternalOutput":
            out_names.append(name)
            shape = tuple(alloc.tensor_shape)
            dtype = mybir.dt.np(alloc.dtype)
            out_avals.append(jax.core.ShapedArray(shape, dtype))
            zero_outs.append(np.zeros(shape, dtype))
    n_params = len(in_names)
    all_in_names = list(in_names) + out_names
    if partition_name is not None:
        all_in_names.append(partition_name)

    def _body(*args):
        operands = list(args)
        if partition_name is not None:
            operands.append(bass2jax.partition_id_tensor())
        return tuple(
            bass2jax._bass_exec_p.bind(
                *operands,
                out_avals=tuple(out_avals),
                in_names=tuple(all_in_names),
                out_names=tuple(out_names),
                lowering_input_output_aliases=(),
                sim_require_finite=True,
                sim_require_nnan=True,
                nc=nc,
            )
        )

    devices = jax.devices()[:NCORES]
    mesh = Mesh(np.asarray(devices), ("core",))
    in_specs = (PartitionSpec("core"),) * (n_params + len(out_names))
    out_specs = (PartitionSpec("core"),) * len(out_names)
    sharded = jax.jit(
        shard_map(
            _body, mesh=mesh, in_specs=in_specs, out_specs=out_specs, check_rep=False
        ),
        keep_unused=True,
    )
    sh = NamedSharding(mesh, PartitionSpec("core"))
    zeros_dev = [
        jax.device_put(np.zeros((NCORES * z.shape[0], *z.shape[1:]), z.dtype), sh)
        for z in zero_outs
    ]

    def run(in_maps):
        ins_dev = [
            jax.device_put(
                np.concatenate([in_maps[c][n] for c in range(NCORES)], axis=0), sh
            )
            for n in in_names
        ]
        outs = sharded(*ins_dev, *zeros_dev)
        jax.block_until_ready(outs)
        # one output tensor: per-core [PPC, H, W] concatenated on axis 0
        return np.asarray(outs[0])

    _cache["runner"] = run
    return run


def kernel(latent, weights, window_size):
    r = int(window_size)
    assert r == R, f"kernel hardcoded for window_size={R}, got {r}"

    run = _get_runner()
    in_maps = _prep_inputs(latent, weights)
    full = run(in_maps)
    return full.reshape(B, C, H, W).astype(np.float32, copy=False)

